# revision 1
# baseline (speedup 1.0000x reference)
"""MoE (MiMo-V2) kernel for 8x Trainium2 NeuronCores.

Strategy (expert-parallel, per the sharding hint):
  - Host: grouped-topk routing (exact replica of the reference gate, run in
    fp32 on jax-cpu), then tokens are gathered per expert into fixed-capacity
    segments. Each of the 8 cores owns 8 experts.
  - Device (Bass/Tile, one SPMD program): for each local expert, stream its
    gathered tokens through gate/up matmuls (bf16 operands, fp32 PSUM
    accumulate), silu*mul on ACT/DVE, down matmul back to token-major
    layout, scale rows by the combine weights, write gathered rows out.
  - Host: scatter-add the gathered per-expert rows into the [T, H] output.
"""

import numpy as np
import ml_dtypes

T, H, E, I, K, G, KG = 16384, 1024, 64, 768, 8, 8, 4
P = 128
NCORES = 8
EPC = E // NCORES  # experts per core
HC = H // P  # 8 contraction chunks for gate/up
IC = I // P  # 6 contraction chunks for down
I2 = 2 * I  # fused gate+up output width

BF16 = ml_dtypes.bfloat16

_program_cache = {}
last_results = None  # BassKernelResults of the most recent launch (for test.py)


def _routing(hidden, gate_w, bias):
    """Exact replica of reference._grouped_topk on jax-cpu (fp32)."""
    import jax
    import jax.numpy as jnp

    cpu = jax.devices("cpu")[0]
    with jax.default_device(cpu):
        hidden = jnp.asarray(np.asarray(hidden), jnp.float32)
        gate_w = jnp.asarray(np.asarray(gate_w), jnp.float32)
        bias = jnp.asarray(np.asarray(bias), jnp.float32)
        logits = hidden @ gate_w.T
        scores = jax.nn.sigmoid(logits)
        s_choice = scores + bias[None, :]
        t, e = scores.shape
        grouped = s_choice.reshape(t, G, e // G)
        top2, _ = jax.lax.top_k(grouped, 2)
        group_scores = top2.sum(-1)
        _, gidx = jax.lax.top_k(group_scores, KG)
        gmask = jnp.zeros((t, G), jnp.float32).at[jnp.arange(t)[:, None], gidx].set(1.0)
        emask = jnp.repeat(gmask, e // G, axis=1)
        masked = jnp.where(emask > 0, s_choice, -jnp.inf)
        _, topk_idx = jax.lax.top_k(masked, K)
        topk_w = jnp.take_along_axis(scores, topk_idx, axis=1)
        topk_w = topk_w / (topk_w.sum(-1, keepdims=True) + 1e-20)
        return np.asarray(topk_idx), np.asarray(topk_w, np.float32)


def _build_program(blocks):
    """One SPMD Bass program; every expert segment has capacity C=sum(blocks)."""
    import concourse.mybir as mybir
    from concourse import bacc
    from concourse.tile import TileContext

    C = sum(blocks)
    NC = EPC * C
    bf = mybir.dt.bfloat16
    f32 = mybir.dt.float32
    Silu = mybir.ActivationFunctionType.Silu
    mult = mybir.AluOpType.mult

    nc = bacc.Bacc("TRN2", target_bir_lowering=False, debug=False, num_devices=NCORES)
    xgt = nc.dram_tensor("xgt", [H, NC], bf, kind="ExternalInput").ap()
    wgu = nc.dram_tensor("wgu", [EPC, H, I2], bf, kind="ExternalInput").ap()
    wd = nc.dram_tensor("wd", [EPC, I, H], bf, kind="ExternalInput").ap()
    cv = nc.dram_tensor("cv", [NC, 1], f32, kind="ExternalInput").ap()
    g = nc.dram_tensor("g", [NC, H], f32, kind="ExternalOutput").ap()

    with TileContext(nc) as tc:
        with (
            tc.tile_pool(name="wpool", bufs=2) as wpool,
            tc.tile_pool(name="xpool", bufs=3) as xpool,
            tc.tile_pool(name="apool", bufs=2) as apool,
            tc.tile_pool(name="spool", bufs=3) as spool,
            tc.tile_pool(name="opool", bufs=4) as opool,
            tc.tile_pool(name="cpool", bufs=4) as cpool,
            tc.tile_pool(name="psg", bufs=2, space="PSUM") as psg,
            tc.tile_pool(name="psu", bufs=2, space="PSUM") as psu,
            tc.tile_pool(name="pso", bufs=4, space="PSUM") as pso,
        ):
            xgt_r = xgt.rearrange("(c p) t -> p c t", p=P)  # [128, HC, NC]
            for ei in range(EPC):
                wgu_sb = wpool.tile([P, HC, I2], bf, tag="wgu")
                nc.sync.dma_start(
                    out=wgu_sb[:], in_=wgu[ei].rearrange("(c p) i -> p c i", p=P)
                )
                wd_sb = wpool.tile([P, IC, H], bf, tag="wd")
                nc.sync.dma_start(
                    out=wd_sb[:], in_=wd[ei].rearrange("(c p) h -> p c h", p=P)
                )
                off = 0
                for bn in blocks:
                    s = ei * C + off
                    xg_sb = xpool.tile([P, HC, 512], bf, tag="xg")
                    nc.sync.dma_start(
                        out=xg_sb[:, :, :bn], in_=xgt_r[:, :, s : s + bn]
                    )
                    act_sb = apool.tile([P, IC, 512], bf, tag="act")
                    for i in range(IC):
                        pg = psg.tile([P, 512], f32, tag="pg")
                        pu = psu.tile([P, 512], f32, tag="pu")
                        for hc in range(HC):
                            nc.tensor.matmul(
                                out=pg[:, :bn],
                                lhsT=wgu_sb[:, hc, i * P : (i + 1) * P],
                                rhs=xg_sb[:, hc, :bn],
                                start=(hc == 0),
                                stop=(hc == HC - 1),
                            )
                        for hc in range(HC):
                            nc.tensor.matmul(
                                out=pu[:, :bn],
                                lhsT=wgu_sb[:, hc, I + i * P : I + (i + 1) * P],
                                rhs=xg_sb[:, hc, :bn],
                                start=(hc == 0),
                                stop=(hc == HC - 1),
                            )
                        sg = spool.tile([P, 512], f32, tag="sg")
                        nc.scalar.activation(out=sg[:, :bn], in_=pg[:, :bn], func=Silu)
                        nc.vector.tensor_tensor(
                            out=act_sb[:, i, :bn], in0=sg[:, :bn], in1=pu[:, :bn], op=mult
                        )
                    for ts in range(bn // P):
                        ct = cpool.tile([P, 1], f32, tag="ct")
                        nc.sync.dma_start(
                            out=ct[:], in_=cv[s + ts * P : s + (ts + 1) * P, :]
                        )
                        for nh in range(2):
                            po = pso.tile([P, 512], f32, tag="po")
                            for i in range(IC):
                                nc.tensor.matmul(
                                    out=po[:],
                                    lhsT=act_sb[:, i, ts * P : (ts + 1) * P],
                                    rhs=wd_sb[:, i, nh * 512 : (nh + 1) * 512],
                                    start=(i == 0),
                                    stop=(i == IC - 1),
                                )
                            ob = opool.tile([P, 512], f32, tag="ob")
                            nc.vector.tensor_tensor(
                                out=ob[:],
                                in0=po[:],
                                in1=ct[:].to_broadcast([P, 512]),
                                op=mult,
                            )
                            nc.sync.dma_start(
                                out=g[
                                    s + ts * P : s + (ts + 1) * P,
                                    nh * 512 : (nh + 1) * 512,
                                ],
                                in_=ob[:],
                            )
                    off += bn
    nc.compile()
    return nc


def kernel(hidden_states, gate_weight, correction_bias, w_gate, w_up, w_down):
    global last_results
    from concourse.bass_utils import run_bass_kernel_spmd

    hidden = np.ascontiguousarray(np.asarray(hidden_states, np.float32))
    w_gate = np.asarray(w_gate, np.float32)
    w_up = np.asarray(w_up, np.float32)
    w_down = np.asarray(w_down, np.float32)

    topk_idx, topk_w = _routing(hidden, gate_weight, correction_bias)

    # Per-expert token lists (ascending), via stable sort of the (token, k) pairs.
    flat_e = topk_idx.ravel()
    order = np.argsort(flat_e, kind="stable")
    tokens_sorted = (order // K).astype(np.int64)
    weights_sorted = topk_w.ravel()[order]
    counts = np.bincount(flat_e, minlength=E)
    starts = np.zeros(E + 1, np.int64)
    np.cumsum(counts, out=starts[1:])

    cap = int(counts.max())
    C = ((cap + P - 1) // P) * P
    blocks = [512] * (C // 512)
    if C % 512:
        blocks.append(C % 512)

    key = tuple(blocks)
    if key not in _program_cache:
        _program_cache[key] = _build_program(blocks)
    nc = _program_cache[key]

    in_maps = []
    tok_lists = []
    for c in range(NCORES):
        perm = np.zeros(EPC * C, np.int64)
        cw = np.zeros((EPC * C, 1), np.float32)
        toks_c = []
        for j in range(EPC):
            e = c * EPC + j
            n = counts[e]
            te = tokens_sorted[starts[e] : starts[e] + n]
            perm[j * C : j * C + n] = te
            cw[j * C : j * C + n, 0] = weights_sorted[starts[e] : starts[e] + n]
            toks_c.append(te)
        tok_lists.append(toks_c)
        xgt = np.ascontiguousarray(hidden[perm].T).astype(BF16)
        wgu_c = np.empty((EPC, H, I2), BF16)
        wd_c = np.empty((EPC, I, H), BF16)
        for j in range(EPC):
            e = c * EPC + j
            wgu_c[j, :, :I] = w_gate[e].T.astype(BF16)
            wgu_c[j, :, I:] = w_up[e].T.astype(BF16)
            wd_c[j] = w_down[e].T.astype(BF16)
        in_maps.append({"xgt": xgt, "wgu": wgu_c, "wd": wd_c, "cv": cw})

    last_results = run_bass_kernel_spmd(nc, in_maps, list(range(NCORES)))

    out = np.zeros((T, H), np.float32)
    for c in range(NCORES):
        gc = last_results.results[c]["g"]
        for j in range(EPC):
            e = c * EPC + j
            n = counts[e]
            out[tok_lists[c][j]] += gc[j * C : j * C + n]
    return out
